# revision 1
# baseline (speedup 1.0000x reference)
"""MHA Bass kernel for TRN2, 8 NeuronCores.

Sharding: data-parallel on batch (2) x tensor-parallel on heads (4 groups of 4
heads). Core c handles batch c//4 and heads 4*(c%4)..4*(c%4)+3 (columns
m0=256*(c%4)). LayerNorm is folded into the projections as a rank-1
correction (gamma folded into weights on host; mean/var computed on-device
via ones-matmuls); attention computed with transposed scores (S^T) so the
softmax'd P^T feeds the O-matmul directly; softmax denominators ride the
O-matmul as a ones-row (M=65); out-projection partials are ReduceScattered
across each batch group of 4 cores.
"""
import numpy as np

B, LQ, D = 2, 2048, 1024
NHEAD, DHEAD = 16, 64
NC = 8
GPC = 4              # cores per batch group
MPC = 256            # output cols per core
N_DCH = D // 128     # 8 d-chunks
N_TCH = LQ // 128    # 16 token chunks
N_TT = LQ // 512     # 4 token tiles of 512
HPC = 4              # heads per core

_NC_CACHE = [None]


def _build():
    import concourse.bacc as bacc
    import concourse.mybir as mybir
    from concourse import tile

    f32, bf16 = mybir.dt.float32, mybir.dt.bfloat16
    AF = mybir.ActivationFunctionType
    MUL, ADD, SUB = mybir.AluOpType.mult, mybir.AluOpType.add, mybir.AluOpType.subtract

    nc = bacc.Bacc("TRN2", target_bir_lowering=False, debug=False, num_devices=NC)

    xq = nc.dram_tensor("xq", [LQ, D], f32, kind="ExternalInput").ap()
    xk = nc.dram_tensor("xk", [LQ, D], f32, kind="ExternalInput").ap()
    xv = nc.dram_tensor("xv", [LQ, D], f32, kind="ExternalInput").ap()
    wqT = nc.dram_tensor("wqT", [D, MPC], bf16, kind="ExternalInput").ap()
    wkT = nc.dram_tensor("wkT", [D, MPC], bf16, kind="ExternalInput").ap()
    wvT = nc.dram_tensor("wvT", [D, MPC], bf16, kind="ExternalInput").ap()
    wgT = nc.dram_tensor("wgT", [D, MPC], bf16, kind="ExternalInput").ap()
    woT = nc.dram_tensor("woT", [MPC, D], bf16, kind="ExternalInput").ap()
    mucq = nc.dram_tensor("mucq", [1, MPC], bf16, kind="ExternalInput").ap()
    muck = nc.dram_tensor("muck", [1, MPC], bf16, kind="ExternalInput").ap()
    mucv = nc.dram_tensor("mucv", [1, MPC], bf16, kind="ExternalInput").ap()
    mucg = nc.dram_tensor("mucg", [1, MPC], bf16, kind="ExternalInput").ap()
    bq_d = nc.dram_tensor("bq", [MPC], f32, kind="ExternalInput").ap()
    bk_d = nc.dram_tensor("bk", [MPC], f32, kind="ExternalInput").ap()
    bv_d = nc.dram_tensor("bv", [MPC], f32, kind="ExternalInput").ap()
    bg_d = nc.dram_tensor("bgt", [MPC], f32, kind="ExternalInput").ap()
    ident = nc.dram_tensor("ident", [128, 128], bf16, kind="ExternalInput").ap()
    out_d = nc.dram_tensor("out", [MPC, LQ], f32, kind="ExternalOutput").ap()

    EPS = 1024.0 * 1024.0 * 1e-5

    with tile.TileContext(nc) as tc:
        import contextlib
        es = contextlib.ExitStack()
        with es:
            const = es.enter_context(tc.tile_pool(name="const", bufs=1))
            persist = es.enter_context(tc.tile_pool(name="persist", bufs=1))

            ones = const.tile([128, 128], bf16)
            nc.gpsimd.memset(ones[:, :], 1.0)
            eps_t = const.tile([128, 1], f32)
            nc.gpsimd.memset(eps_t[:, :], 1e-5)
            idt = const.tile([128, 128], bf16)
            nc.sync.dma_start(out=idt[:, :], in_=ident[:, :])

            # weights: [128, 8, 256] layouts (d-chunk, cols)
            wts = {}
            for nm, dr in (("q", wqT), ("k", wkT), ("v", wvT), ("g", wgT)):
                t = const.tile([128, N_DCH, MPC], bf16, tag=f"w{nm}")
                for j in range(N_DCH):
                    nc.sync.dma_start(out=t[:, j, :], in_=dr[128 * j:128 * (j + 1), :])
                wts[nm] = t
            wo_t = const.tile([128, 2, D], bf16)
            for mc in range(2):
                nc.sync.dma_start(out=wo_t[:, mc, :], in_=woT[128 * mc:128 * (mc + 1), :])
            mucs = {}
            for nm, dr in (("q", mucq), ("k", muck), ("v", mucv), ("g", mucg)):
                t = const.tile([1, MPC], bf16, tag=f"muc{nm}")
                nc.sync.dma_start(out=t[:, :], in_=dr[:, :])
                mucs[nm] = t
            biases = {}
            for nm, dr in (("q", bq_d), ("k", bk_d), ("v", bv_d), ("g", bg_d)):
                t = const.tile([128, 2], f32, tag=f"b{nm}")
                nc.sync.dma_start(out=t[:, :], in_=dr.rearrange("(c p) -> p c", p=128))
                biases[nm] = t

            # persistent activation outputs
            qhT = persist.tile([128, 2, LQ], bf16, tag="qhT")
            khT = persist.tile([128, 2, LQ], bf16, tag="khT")
            gT = persist.tile([128, 2, LQ], bf16, tag="gT")
            ygT = persist.tile([128, 2, LQ], bf16, tag="ygT")
            vaug = persist.tile([128, N_TCH, HPC, 65], bf16, tag="vaug")
            # contiguous memset; v-transposes later overwrite cols 0:64, col 64 stays 1.0
            nc.gpsimd.memset(vaug[:, :, :, :], 1.0)

            with tc.tile_pool(name="ph1", bufs=2) as ph1, \
                 tc.tile_pool(name="ph1b", bufs=1) as ph1b, \
                 tc.tile_pool(name="stage", bufs=1) as stage, \
                 tc.tile_pool(name="scr", bufs=2) as scr, \
                 tc.tile_pool(name="ps1", bufs=2, space="PSUM") as ps1, \
                 tc.tile_pool(name="pstr", bufs=2, space="PSUM") as pstr:

                def load_T(x_dram):
                    """load [2048,1024] f32 -> bf16 transposed xT [128,(i,j,t)]."""
                    xT = ph1.tile([128, N_TCH, N_DCH, 128], bf16, tag="xT")
                    xr = stage.tile([128, N_TCH, D], bf16, tag="xrow")
                    # 4 cast-DMAs (2.1MB each): out[p, i, c] = x[512q + 128i + p, c]
                    for qq in range(4):
                        nc.gpsimd.dma_start(
                            out=xr[:, 4 * qq:4 * (qq + 1), :],
                            in_=x_dram[512 * qq:512 * (qq + 1), :].rearrange(
                                "(i p) c -> p i c", p=128))
                        for i in range(4 * qq, 4 * qq + 4):
                            nc.sync.dma_start(out=xT[:, i, :, :], in_=xr[:, i, :], transpose=True)
                    return xT

                def rhs_slice(xT, j, tt):
                    return xT[:, 4 * tt:4 * (tt + 1), j, :]

                def stats(xT):
                    """returns r_rep [128,2048] f32 (1/sqrt(n*S2-S1^2+n^2 eps)), mu [1,2048] bf16"""
                    r_rep = ph1b.tile([128, LQ], f32, tag="r_rep")
                    mu = ph1b.tile([1, LQ], bf16, tag="mu")
                    for tt in range(N_TT):
                        sl = slice(512 * tt, 512 * (tt + 1))
                        s1 = ps1.tile([128, 512], f32, tag="s1")
                        s2 = ps1.tile([128, 512], f32, tag="s2")
                        for j in range(N_DCH):
                            nc.tensor.matmul(s1[:, :], ones[:, :], rhs_slice(xT, j, tt),
                                             start=(j == 0), stop=(j == N_DCH - 1))
                        for j in range(N_DCH):
                            sq = scr.tile([128, 512], bf16, tag="sq")
                            nc.vector.tensor_mul(sq[:, :], rhs_slice(xT, j, tt), rhs_slice(xT, j, tt))
                            nc.tensor.matmul(s2[:, :], ones[:, :], sq[:, :],
                                             start=(j == 0), stop=(j == N_DCH - 1))
                        s1s = scr.tile([128, 512], f32, tag="s1s")
                        nc.vector.tensor_copy(s1s[:, :], s1[:, :])
                        t1 = scr.tile([128, 512], f32, tag="t1")
                        nc.vector.tensor_mul(t1[:, :], s1s[:, :], s1s[:, :])
                        t2 = scr.tile([128, 512], f32, tag="t2")
                        nc.vector.scalar_tensor_tensor(t2[:, :], s2[:, :], 1024.0, t1[:, :], MUL, SUB)
                        t3 = scr.tile([128, 512], f32, tag="t3")
                        nc.scalar.activation(t3[:, :], t2[:, :], AF.Sqrt, bias=eps_t[:, :], scale=1.0 / (1024.0 * 1024.0))
                        nc.vector.reciprocal(r_rep[:, sl], t3[:, :])
                        nc.vector.tensor_scalar(mu[0:1, sl], s1s[0:1, :], 1.0 / 1024.0, None, op0=MUL)
                    return r_rep, mu

                def project(xT, wkey, muckey, mu, r_rep, out_t, sigmoid=False):
                    """out_t[:, mc, :] (bf16) = drain((x-mu)@W'^T * r) [*1024 + bias]"""
                    w = wts[wkey]
                    mc_t = mucs[muckey]
                    bias = biases[muckey if not sigmoid else "g"]
                    for mc in range(2):
                        for tt in range(N_TT):
                            sl = slice(512 * tt, 512 * (tt + 1))
                            pp = ps1.tile([128, 512], f32, tag="pp")
                            for j in range(N_DCH):
                                nc.tensor.matmul(pp[:, :], w[:, j, 128 * mc:128 * (mc + 1)],
                                                 rhs_slice(xT, j, tt), start=(j == 0), stop=False)
                            nc.tensor.matmul(pp[:, :], mc_t[:, 128 * mc:128 * (mc + 1)],
                                             mu[0:1, sl], start=False, stop=True)
                            if sigmoid:
                                tmp = scr.tile([128, 512], f32, tag="ptmp")
                                nc.vector.tensor_mul(tmp[:, :], pp[:, :], r_rep[:, sl])
                                nc.scalar.activation(out_t[:, mc, sl], tmp[:, :], AF.Sigmoid,
                                                     bias=bias[:, mc:mc + 1], scale=1.0)
                            else:
                                nc.vector.scalar_tensor_tensor(
                                    out_t[:, mc, sl], pp[:, :], bias[:, mc:mc + 1],
                                    r_rep[:, sl], ADD, MUL)

                # ---- q ----
                xT = load_T(xq)
                r_rep, mu = stats(xT)
                project(xT, "q", "q", mu, r_rep, qhT)
                project(xT, "g", "g", mu, r_rep, gT, sigmoid=True)
                # ---- k ----
                xT = load_T(xk)
                r_rep, mu = stats(xT)
                project(xT, "k", "k", mu, r_rep, khT)
                # ---- v ----
                xT = load_T(xv)
                r_rep, mu = stats(xT)
                vhT = ph1b.tile([128, 2, LQ], bf16, tag="vhT")
                project(xT, "v", "v", mu, r_rep, vhT)
                # transpose vhT -> vaug
                for mc in range(2):
                    for s in range(N_TCH):
                        pt = pstr.tile([128, 128], bf16, tag="ptr")
                        nc.tensor.transpose(pt[:, :], vhT[:, mc, 128 * s:128 * (s + 1)], idt[:, :])
                        nc.vector.tensor_copy(vaug[:, s, 2 * mc, 0:64], pt[:, 0:64])
                        nc.vector.tensor_copy(vaug[:, s, 2 * mc + 1, 0:64], pt[:, 64:128])

            # ---- attention: head pairs; 4 independent (head x tt) chains of
            # [128,512] ST -> exp -> O for fine-grained PE/ACT pipelining
            with tc.tile_pool(name="att", bufs=2) as att, \
                 tc.tile_pool(name="ps_st", bufs=1, space="PSUM") as ps_st, \
                 tc.tile_pool(name="ps_o", bufs=1, space="PSUM") as ps_o:
                for hp in range(2):
                    kc = hp
                    for th in range(2):
                        t0 = 1024 * th
                        o_ps = {}
                        for hb in range(2):
                            for tt in range(2):
                                o_ps[hb, tt] = ps_o.tile([65, 512], f32, name=f"o{hb}{tt}", tag=f"o{hb}{tt}")
                        for s in range(N_TCH):
                            for hb in range(2):
                                r0 = 64 * hb
                                for tt in range(2):
                                    sl_t = slice(t0 + 512 * tt, t0 + 512 * (tt + 1))
                                    st = ps_st.tile([128, 512], f32, tag=f"st{hb}{tt}")
                                    nc.tensor.matmul(st[:, :],
                                                     khT[r0:r0 + 64, kc, 128 * s:128 * (s + 1)],
                                                     qhT[r0:r0 + 64, kc, sl_t],
                                                     start=True, stop=True)
                                    pt = att.tile([128, 512], bf16, tag=f"pt{hb}{tt}")
                                    nc.scalar.activation(pt[:, :], st[:, :], AF.Exp, scale=0.125)
                                    nc.tensor.matmul(o_ps[hb, tt][:, :],
                                                     vaug[:, s, 2 * hp + hb, :], pt[:, :],
                                                     start=(s == 0), stop=(s == N_TCH - 1))
                        for hb in range(2):
                            r0 = 64 * hb
                            for tt in range(2):
                                o_p = o_ps[hb, tt]
                                sl_y = slice(t0 + 512 * tt, t0 + 512 * (tt + 1))
                                li_f = att.tile([65, 512], f32, tag="lif")
                                nc.vector.reciprocal(li_f[64:65, :], o_p[64:65, :])
                                li_b = att.tile([65, 512], bf16, tag="lib")
                                nc.vector.tensor_copy(li_b[64:65, :], li_f[64:65, :])
                                bc = ps_st.tile([64, 512], f32, tag=f"st{hb}{tt}")
                                nc.tensor.matmul(bc[:, :], ones[64:65, 0:64],
                                                 li_b[64:65, :], start=True, stop=True)
                                bcs = att.tile([64, 512], f32, tag="bcs")
                                nc.vector.tensor_copy(bcs[:, :], bc[:, :])
                                tmp2 = att.tile([128, 512], f32, tag="tmp2")
                                nc.vector.tensor_mul(tmp2[0:64, :], o_p[0:64, :], bcs[:, :])
                                if r0 != 0:
                                    # partition shift 0->64 via SBUF->SBUF DMA
                                    nc.gpsimd.dma_start(out=tmp2[64:128, :], in_=tmp2[0:64, :])
                                nc.vector.tensor_mul(ygT[r0:r0 + 64, kc, sl_y],
                                                     tmp2[r0:r0 + 64, :],
                                                     gT[r0:r0 + 64, kc, sl_y])

            # ---- out-proj + reduce-scatter ----
            with tc.tile_pool(name="po", bufs=2, space="PSUM") as po_p, \
                 tc.tile_pool(name="od", bufs=4) as od_p, \
                 tc.tile_pool(name="dram", bufs=1, space="DRAM") as dram_p:
                outb = dram_p.tile([D, LQ], f32, tag="outb")
                outrs = dram_p.tile([MPC, LQ], f32, tag="outrs")
                for half in range(4):
                    for nk in range(2 * half, 2 * half + 2):
                        for tt in range(N_TT):
                            po = po_p.tile([128, 512], f32, tag="po")
                            for mc in range(2):
                                nc.tensor.matmul(po[:, :], wo_t[:, mc, 128 * nk:128 * (nk + 1)],
                                                 ygT[:, mc, 512 * tt:512 * (tt + 1)],
                                                 start=(mc == 0), stop=(mc == 1))
                            ot = od_p.tile([128, 512], f32, tag="ot")
                            nc.vector.tensor_copy(ot[:, :], po[:, :])
                            nc.sync.dma_start(
                                out=outb[128 * nk:128 * (nk + 1), 512 * tt:512 * (tt + 1)],
                                in_=ot[:, :])
                    nc.gpsimd.collective_compute(
                        "ReduceScatter", ADD,
                        replica_groups=[[0, 1, 2, 3], [4, 5, 6, 7]],
                        ins=[outb[256 * half:256 * (half + 1), :].opt()],
                        outs=[outrs[64 * half:64 * (half + 1), :].opt()],
                    )
                for ch in range(2):
                    ob = od_p.tile([128, LQ], f32, tag="ob")
                    nc.sync.dma_start(out=ob[:, :], in_=outrs[128 * ch:128 * (ch + 1), :])
                    nc.sync.dma_start(out=out_d[128 * ch:128 * (ch + 1), :], in_=ob[:, :])

    nc.compile()
    return nc


def kernel(q, k, v, qln_g, qln_b, kvln_g, kvln_b, Wq, Wk, Wv, Wg, bg, Wo):
    import concourse.mybir as mybir
    from concourse import bass_utils

    bf16 = mybir.dt.np(mybir.dt.bfloat16)
    q = np.asarray(q, np.float32)
    k = np.asarray(k, np.float32)
    v = np.asarray(v, np.float32)
    qln_g = np.asarray(qln_g, np.float32)
    qln_b = np.asarray(qln_b, np.float32)
    kvln_g = np.asarray(kvln_g, np.float32)
    kvln_b = np.asarray(kvln_b, np.float32)
    Wq, Wk, Wv = np.asarray(Wq, np.float32), np.asarray(Wk, np.float32), np.asarray(Wv, np.float32)
    Wg, Wo = np.asarray(Wg, np.float32), np.asarray(Wo, np.float32)
    bg = np.asarray(bg, np.float32)

    # fold LN gamma into weights; beta into bias vectors
    Wqp, Wgp = Wq * qln_g[None, :], Wg * qln_g[None, :]
    Wkp, Wvp = Wk * kvln_g[None, :], Wv * kvln_g[None, :]
    bq_f, bk_f, bv_f = Wq @ qln_b, Wk @ kvln_b, Wv @ kvln_b
    bg_f = Wg @ qln_b + bg
    idm = np.eye(128, dtype=np.float32)

    if _NC_CACHE[0] is None:
        _NC_CACHE[0] = _build()
    nc = _NC_CACHE[0]

    in_maps = []
    for c in range(NC):
        beta, g = c // GPC, c % GPC
        m0 = MPC * g
        sl = slice(m0, m0 + MPC)
        in_maps.append({
            "xq": q[beta], "xk": k[beta], "xv": v[beta],
            "wqT": Wqp[sl, :].T.astype(bf16), "wkT": Wkp[sl, :].T.astype(bf16),
            "wvT": Wvp[sl, :].T.astype(bf16), "wgT": Wgp[sl, :].T.astype(bf16),
            "woT": Wo[:, sl].T.astype(bf16),
            "mucq": -Wqp[sl, :].sum(1)[None, :].astype(bf16),
            "muck": -Wkp[sl, :].sum(1)[None, :].astype(bf16),
            "mucv": -Wvp[sl, :].sum(1)[None, :].astype(bf16),
            "mucg": -Wgp[sl, :].sum(1)[None, :].astype(bf16),
            "bq": bq_f[sl], "bk": bk_f[sl], "bv": bv_f[sl], "bgt": bg_f[sl],
            "ident": idm.astype(bf16),
        })
    global _last_in_maps
    _last_in_maps = in_maps
    res = bass_utils.run_bass_kernel_spmd(nc, in_maps, core_ids=list(range(NC)))
    out = np.empty((B, LQ, D), np.float32)
    for beta in range(B):
        rows = np.empty((D, LQ), np.float32)
        for qtr in range(4):
            for g in range(GPC):
                rows[256 * qtr + 64 * g:256 * qtr + 64 * (g + 1)] = \
                    res.results[GPC * beta + g]["out"][64 * qtr:64 * (qtr + 1)]
        out[beta] = rows.T
    return out



# revision 22
# speedup vs baseline: 1.4914x; 1.4914x over previous
"""MHA Bass kernel for TRN2, 8 NeuronCores.

Sharding: data-parallel on batch (2) x tensor-parallel on heads (4 groups of 4
heads). Core c handles batch c//4 and heads 4*(c%4)..4*(c%4)+3 (columns
m0=256*(c%4)). Activations arrive host-transposed/bf16 ([d, tokens]) so no
on-device transposes are needed. LayerNorm gamma is folded into the weights;
mean/var come from ones-matmuls; rsqrt on ACT. q/k/g project column-major
(cols on partitions); v projects token-major directly into the augmented
V-transpose layout (ones column rides the O-matmul to produce softmax
denominators). Attention runs 4 chains (head x half) per 512-token quarter
with software-pipelined ST->exp->O so PE and ACT both stream. Out-projection
partials (bf16) ReduceScatter per token quarter straight into the output,
overlapping the collective with the next quarter's compute.
"""
import numpy as np

B, LQ, D = 2, 2048, 1024
NHEAD, DHEAD = 16, 64
NC = 8
GPC = 4              # cores per batch group
MPC = 256            # output cols per core
N_DCH = D // 128     # 8 d-chunks
N_SCH = LQ // 128    # 16 token chunks of 128
N_TT = LQ // 512     # 4 token tiles of 512

_NC_CACHE = [None]


def _build(debug=False):
    import concourse.bacc as bacc
    import concourse.mybir as mybir
    from concourse import tile

    f32, bf16 = mybir.dt.float32, mybir.dt.bfloat16
    AF = mybir.ActivationFunctionType
    MUL, ADD, SUB = mybir.AluOpType.mult, mybir.AluOpType.add, mybir.AluOpType.subtract

    nc = bacc.Bacc("TRN2", target_bir_lowering=False, debug=False, num_devices=NC)

    xqT = nc.dram_tensor("xqT", [D, LQ], bf16, kind="ExternalInput").ap()
    xkT = nc.dram_tensor("xkT", [D, LQ], bf16, kind="ExternalInput").ap()
    xvT = nc.dram_tensor("xvT", [D, LQ], bf16, kind="ExternalInput").ap()
    wqT = nc.dram_tensor("wqT", [D, MPC], bf16, kind="ExternalInput").ap()
    wkT = nc.dram_tensor("wkT", [D, MPC], bf16, kind="ExternalInput").ap()
    wvT = nc.dram_tensor("wvT", [D, MPC], bf16, kind="ExternalInput").ap()
    wgT = nc.dram_tensor("wgT", [D, MPC], bf16, kind="ExternalInput").ap()
    woT = nc.dram_tensor("woT", [MPC, D], bf16, kind="ExternalInput").ap()
    mucq = nc.dram_tensor("mucq", [1, MPC], bf16, kind="ExternalInput").ap()
    muck = nc.dram_tensor("muck", [1, MPC], bf16, kind="ExternalInput").ap()
    mucv = nc.dram_tensor("mucv", [1, MPC], bf16, kind="ExternalInput").ap()
    mucg = nc.dram_tensor("mucg", [1, MPC], bf16, kind="ExternalInput").ap()
    bq_d = nc.dram_tensor("bq", [MPC], f32, kind="ExternalInput").ap()
    bk_d = nc.dram_tensor("bk", [MPC], f32, kind="ExternalInput").ap()
    bg_d = nc.dram_tensor("bgt", [MPC], f32, kind="ExternalInput").ap()
    bvb_d = nc.dram_tensor("bvb", [128, MPC], f32, kind="ExternalInput").ap()
    identf = nc.dram_tensor("identf", [128, 128], f32, kind="ExternalInput").ap()
    out_d = nc.dram_tensor("out", [MPC, LQ], bf16, kind="ExternalOutput").ap()
    if debug:
        dbg = {nm: nc.dram_tensor(f"dbg_{nm}", shp, bf16, kind="ExternalOutput").ap()
               for nm, shp in (("qhT", [128, 2, LQ]), ("khT", [128, 2, LQ]),
                               ("gT", [128, 2, LQ]), ("ygT", [128, 2, LQ]),
                               ("vaug", [128, N_SCH, 4, 65]))}
        dbg["rv"] = nc.dram_tensor("dbg_rv", [128, N_SCH], mybir.dt.float32,
                                   kind="ExternalOutput").ap()

    with tile.TileContext(nc) as tc:
        import contextlib
        es = contextlib.ExitStack()
        with es:
            const = es.enter_context(tc.tile_pool(name="const", bufs=1))
            persist = es.enter_context(tc.tile_pool(name="persist", bufs=1))

            ones = const.tile([128, 128], bf16)
            nc.gpsimd.memset(ones[:, :], 1.0)
            eps_t = const.tile([128, 1], f32)
            nc.gpsimd.memset(eps_t[:, :], 1e-5)
            idf = const.tile([128, 128], f32)
            nc.sync.dma_start(out=idf[:, :], in_=identf[:, :])

            # weights: [128, 8, 256] layouts (d-chunk, cols)
            wts = {}
            for nm, dr in (("q", wqT), ("k", wkT), ("v", wvT), ("g", wgT)):
                t = const.tile([128, N_DCH, MPC], bf16, tag=f"w{nm}")
                for j in range(N_DCH):
                    nc.sync.dma_start(out=t[:, j, :], in_=dr[128 * j:128 * (j + 1), :])
                wts[nm] = t
            wo_t = const.tile([128, 2, D], bf16)
            for kc in range(2):
                nc.sync.dma_start(out=wo_t[:, kc, :], in_=woT[128 * kc:128 * (kc + 1), :])
            mucs = {}
            for nm, dr in (("q", mucq), ("k", muck), ("v", mucv), ("g", mucg)):
                t = const.tile([1, MPC], bf16, tag=f"muc{nm}")
                nc.sync.dma_start(out=t[:, :], in_=dr[:, :])
                mucs[nm] = t
            biases = {}
            for nm, dr in (("q", bq_d), ("k", bk_d), ("g", bg_d)):
                t = const.tile([128, 2], f32, tag=f"b{nm}")
                nc.sync.dma_start(out=t[:, :], in_=dr.rearrange("(c p) -> p c", p=128))
                biases[nm] = t
            bvb = const.tile([128, MPC], f32)
            nc.sync.dma_start(out=bvb[:, :], in_=bvb_d[:, :])

            # persistent activation outputs
            qhT = persist.tile([128, 2, LQ], bf16, tag="qhT")
            khT = persist.tile([128, 2, LQ], bf16, tag="khT")
            gT = persist.tile([128, 2, LQ], bf16, tag="gT")
            ygT = persist.tile([128, 2, LQ], bf16, tag="ygT")
            vaug = persist.tile([128, N_SCH, 4, 65], bf16, tag="vaug")
            # ones column (col 64) for the softmax-denominator row of the
            # O-matmul; cols 0:64 are overwritten by the v-projection.
            nc.gpsimd.memset(vaug[:, :, :, :], 1.0)
            rv_tok = persist.tile([128, N_SCH], f32, tag="rvtok")

            with tc.tile_pool(name="xt", bufs=2) as xt_pool, \
                 tc.tile_pool(name="rr", bufs=2) as r_pool, \
                 tc.tile_pool(name="scr", bufs=2) as scr, \
                 tc.tile_pool(name="ps_st", bufs=2, space="PSUM") as ps_stat, \
                 tc.tile_pool(name="ps_pp", bufs=2, space="PSUM") as ps_pp, \
                 tc.tile_pool(name="ps_tr", bufs=2, space="PSUM") as ps_tr:

                def load_T(x_dram):
                    """straight load of host-transposed x: [128, 8, 2048] bf16."""
                    xT = xt_pool.tile([128, N_DCH, LQ], bf16, tag="xT")
                    for j in range(N_DCH):
                        for h in range(2):
                            nc.sync.dma_start(
                                out=xT[:, j, 1024 * h:1024 * (h + 1)],
                                in_=x_dram[128 * j:128 * (j + 1),
                                           1024 * h:1024 * (h + 1)])
                    return xT

                def stats(xT, want_tok=False, want_rep=True):
                    """r_rep [128,2048] f32 replicated, mu [1,2048] bf16.
                    want_tok: also fill rv_tok [128,16] (token-major rsqrt)."""
                    r_rep = r_pool.tile([128, LQ], f32, tag="r_rep")
                    mu = r_pool.tile([1, LQ], bf16, tag="mu")
                    for tt in range(N_TT):
                        sl = slice(512 * tt, 512 * (tt + 1))
                        s1 = ps_stat.tile([128, 512], f32, tag="s1")
                        s2 = ps_stat.tile([128, 512], f32, tag="s2")
                        for j in range(N_DCH):
                            nc.tensor.matmul(s1[:, :], ones[:, :], xT[:, j, sl],
                                             start=(j == 0), stop=(j == N_DCH - 1))
                        for j in range(N_DCH):
                            sq = scr.tile([128, 512], bf16, tag="sq")
                            nc.vector.tensor_mul(sq[:, :], xT[:, j, sl], xT[:, j, sl])
                            nc.tensor.matmul(s2[:, :], ones[:, :], sq[:, :],
                                             start=(j == 0), stop=(j == N_DCH - 1))
                        s1s = scr.tile([128, 512], f32, tag="s1s")
                        nc.vector.tensor_copy(s1s[:, :], s1[:, :])
                        t1 = scr.tile([128, 512], f32, tag="t1")
                        nc.vector.tensor_mul(t1[:, :], s1s[:, :], s1s[:, :])
                        t2 = scr.tile([128, 512], f32, tag="t2")
                        nc.vector.scalar_tensor_tensor(t2[:, :], s2[:, :], 1024.0,
                                                       t1[:, :], MUL, SUB)
                        if want_rep:
                            # r = 1/sqrt(t2/n^2 + eps)
                            t3 = scr.tile([128, 512], f32, tag="t3")
                            nc.scalar.activation(t3[:, :], t2[:, :], AF.Sqrt,
                                                 bias=eps_t[:, :],
                                                 scale=1.0 / (1024.0 * 1024.0))
                            nc.vector.reciprocal(r_rep[:, sl], t3[:, :])
                        nc.vector.tensor_scalar(mu[0:1, sl], s1s[0:1, :],
                                                1.0 / 1024.0, None, op0=MUL)
                        if want_tok:
                            # token-major r for the v epilogue: transpose t2
                            # 128-chunks (content replicated across partitions)
                            for i in range(4):
                                s = 4 * tt + i
                                tr = ps_tr.tile([128, 128], f32, tag="tr")
                                nc.tensor.transpose(
                                    tr[:, :], t2[:, 128 * i:128 * (i + 1)], idf[:, :])
                                sd = scr.tile([128, 1], f32, tag="sd")
                                nc.scalar.activation(
                                    sd[:, :], tr[:, 0:1], AF.Sqrt,
                                    bias=eps_t[:, :],
                                    scale=1.0 / (1024.0 * 1024.0))
                                nc.vector.reciprocal(
                                    rv_tok[:, s:s + 1], sd[:, :])
                    return r_rep, mu

                def project(xT, wkey, mu, r_rep, out_t, sigmoid=False):
                    """col-major: out_t[:, mc, sl] = (x W'^T + muc x mu) * r + b"""
                    w = wts[wkey]
                    mc_t = mucs[wkey]
                    bias = biases[wkey]
                    for mc in range(2):
                        for tt in range(N_TT):
                            sl = slice(512 * tt, 512 * (tt + 1))
                            pp = ps_pp.tile([128, 512], f32, tag="pp")
                            for j in range(N_DCH):
                                nc.tensor.matmul(pp[:, :],
                                                 w[:, j, 128 * mc:128 * (mc + 1)],
                                                 xT[:, j, sl],
                                                 start=(j == 0), stop=False)
                            nc.tensor.matmul(pp[:, :], mc_t[:, 128 * mc:128 * (mc + 1)],
                                             mu[0:1, sl], start=False, stop=True)
                            if sigmoid:
                                tmp = scr.tile([128, 512], f32, tag="ptmp")
                                nc.vector.tensor_mul(tmp[:, :], pp[:, :], r_rep[:, sl])
                                nc.scalar.activation(out_t[:, mc, sl], tmp[:, :],
                                                     AF.Sigmoid,
                                                     bias=bias[:, mc:mc + 1], scale=1.0)
                            else:
                                nc.vector.scalar_tensor_tensor(
                                    out_t[:, mc, sl], pp[:, :], bias[:, mc:mc + 1],
                                    r_rep[:, sl], ADD, MUL)

                def project_v(xT, mu):
                    """token-major: vaug[:, s, h, 0:64] = (x W'^T + mu x muc)*r + b"""
                    w = wts["v"]
                    mc_t = mucs["v"]
                    for s in range(N_SCH):
                        ts = slice(128 * s, 128 * (s + 1))
                        pv_full = ps_pp.tile([128, 512], f32, tag="pp")
                        pv = pv_full[:, 0:MPC]
                        for j in range(N_DCH):
                            nc.tensor.matmul(pv, xT[:, j, ts], w[:, j, :],
                                             start=(j == 0), stop=False)
                        nc.tensor.matmul(pv, mu[0:1, ts], mc_t[:, :],
                                         start=False, stop=True)
                        nc.vector.scalar_tensor_tensor(
                            vaug[:, s, :, 0:64],
                            pv, rv_tok[:, s:s + 1], bvb[:, :], MUL, ADD)

                # ---- phase A ----
                xT_q = load_T(xqT)
                xT_k = load_T(xkT)
                rq, muq = stats(xT_q)
                rk, muk = stats(xT_k)
                project(xT_q, "q", muq, rq, qhT)
                project(xT_q, "g", muq, rq, gT, sigmoid=True)
                xT_v = load_T(xvT)      # reuses xT_q's buffer (bufs=2)
                project(xT_k, "k", muk, rk, khT)
                rv, muv = stats(xT_v, want_tok=True, want_rep=False)
                project_v(xT_v, muv)

            # ---- attention + out-proj + per-quarter reduce-scatter ----
            with tc.tile_pool(name="att", bufs=2) as att, \
                 tc.tile_pool(name="ps_a", bufs=1, space="PSUM") as ps_a, \
                 tc.tile_pool(name="ps_o", bufs=1, space="PSUM") as ps_o, \
                 tc.tile_pool(name="dram", bufs=1, space="DRAM") as dram_p:
                outb = dram_p.tile([N_TT, D, 512], bf16, tag="outb")
                outrs = dram_p.tile([N_TT, MPC, 512], bf16, tag="outrs")
                for qt in range(N_TT):
                    t0 = 512 * qt
                    sl_t = slice(t0, t0 + 512)
                    o_ps = {}
                    for hp in range(2):
                        for hb in range(2):
                            o_ps[hp, hb] = ps_o.tile([65, 512], f32,
                                                     name=f"o{hp}{hb}",
                                                     tag=f"o{hp}{hb}")
                    pts = [None, None]
                    # software pipeline: ST(s) then O(s-1) keeps PE busy while
                    # ACT computes exp(s)
                    for s in range(N_SCH + 1):
                        prev = pts
                        if s < N_SCH:
                            pts = [None, None]
                            for hp in range(2):
                                st = ps_a.tile([128, 1024], f32, tag=f"A{hp}")
                                for hb in range(2):
                                    nc.tensor.matmul(
                                        st[:, 512 * hb:512 * (hb + 1)],
                                        khT[64 * hb:64 * (hb + 1), hp,
                                            128 * s:128 * (s + 1)],
                                        qhT[64 * hb:64 * (hb + 1), hp, sl_t],
                                        start=True, stop=True)
                                pt = att.tile([128, 1024], bf16, tag=f"pt{hp}")
                                nc.scalar.activation(pt[:, :], st[:, :], AF.Exp,
                                                     scale=0.125)
                                pts[hp] = pt
                        if s > 0:
                            sp = s - 1
                            for hp in range(2):
                                for hb in range(2):
                                    nc.tensor.matmul(
                                        o_ps[hp, hb][:, :],
                                        vaug[:, sp, 2 * hp + hb, :],
                                        prev[hp][:, 512 * hb:512 * (hb + 1)],
                                        start=(sp == 0), stop=(sp == N_SCH - 1))
                    # epilogue: y * g / denom -> ygT
                    for hp in range(2):
                        for hb in range(2):
                            o_p = o_ps[hp, hb]
                            # denom (row 64) -> bf16, broadcast to 64 rows via
                            # ones-matmul, reciprocal on DVE, then scale y
                            li = att.tile([65, 512], bf16, tag="li")
                            nc.scalar.copy(li[64:65, :], o_p[64:65, :])
                            bc_full = ps_a.tile([128, 1024], f32,
                                                name="bc_full",
                                                tag=f"A{(2 * hp + hb) % 2}")
                            bc = bc_full[0:64, 0:512]
                            nc.tensor.matmul(bc, ones[64:65, 0:64],
                                             li[64:65, :], start=True, stop=True)
                            rec = att.tile([64, 512], f32, tag="rec")
                            nc.vector.reciprocal(rec[:, :], bc)
                            tmp = att.tile([128, 512], bf16, tag="tmp")
                            nc.vector.tensor_mul(tmp[0:64, :], o_p[0:64, :],
                                                 rec[:, :])
                            if hb == 0:
                                nc.gpsimd.tensor_mul(ygT[0:64, hp, sl_t],
                                                     tmp[0:64, :],
                                                     gT[0:64, hp, sl_t])
                            else:
                                # partition shift 0->64 via SBUF->SBUF DMA
                                nc.gpsimd.dma_start(out=tmp[64:128, :],
                                                    in_=tmp[0:64, :])
                                nc.gpsimd.tensor_mul(ygT[64:128, hp, sl_t],
                                                     tmp[64:128, :],
                                                     gT[64:128, hp, sl_t])
                    # out-projection for this token quarter
                    for nk in range(N_DCH):
                        po_full = ps_a.tile([128, 1024], f32, tag=f"A{nk % 2}")
                        po = po_full[:, 0:512]
                        for kc in range(2):
                            nc.tensor.matmul(po, wo_t[:, kc, 128 * nk:128 * (nk + 1)],
                                             ygT[:, kc, sl_t],
                                             start=(kc == 0), stop=(kc == 1))
                        ot = att.tile([128, 512], bf16, tag="ot")
                        if nk % 2 == 0:
                            nc.vector.tensor_copy(ot[:, :], po)
                        else:
                            nc.scalar.copy(ot[:, :], po)
                        nc.sync.dma_start(
                            out=outb[qt, 128 * nk:128 * (nk + 1), :], in_=ot[:, :])
                    nc.gpsimd.collective_compute(
                        "ReduceScatter", ADD,
                        replica_groups=[[0, 1, 2, 3], [4, 5, 6, 7]],
                        ins=[outb[qt, :, :].opt()],
                        outs=[outrs[qt, :, :].opt()],
                    )
                    ob = att.tile([128, 2, 512], bf16, tag="ob")
                    for r in range(2):
                        nc.sync.dma_start(out=ob[:, r, :],
                                          in_=outrs[qt, 128 * r:128 * (r + 1), :])
                        nc.sync.dma_start(out=out_d[128 * r:128 * (r + 1), sl_t],
                                          in_=ob[:, r, :])
                if debug:
                    for nm, t in (("qhT", qhT), ("khT", khT), ("gT", gT),
                                  ("ygT", ygT)):
                        nc.sync.dma_start(out=dbg[nm][:, :, :], in_=t[:, :, :])
                    nc.sync.dma_start(out=dbg["vaug"][:, :, :, :],
                                      in_=vaug[:, :, :, :])
                    nc.sync.dma_start(out=dbg["rv"][:, :], in_=rv_tok[:, :])

    nc.compile()
    return nc


def kernel(q, k, v, qln_g, qln_b, kvln_g, kvln_b, Wq, Wk, Wv, Wg, bg, Wo):
    import concourse.mybir as mybir
    from concourse import bass_utils

    bf16 = mybir.dt.np(mybir.dt.bfloat16)
    q = np.asarray(q, np.float32)
    k = np.asarray(k, np.float32)
    v = np.asarray(v, np.float32)
    qln_g = np.asarray(qln_g, np.float32)
    qln_b = np.asarray(qln_b, np.float32)
    kvln_g = np.asarray(kvln_g, np.float32)
    kvln_b = np.asarray(kvln_b, np.float32)
    Wq, Wk, Wv = np.asarray(Wq, np.float32), np.asarray(Wk, np.float32), np.asarray(Wv, np.float32)
    Wg, Wo = np.asarray(Wg, np.float32), np.asarray(Wo, np.float32)
    bg = np.asarray(bg, np.float32)

    # fold LN gamma into weights; beta into bias vectors
    Wqp, Wgp = Wq * qln_g[None, :], Wg * qln_g[None, :]
    Wkp, Wvp = Wk * kvln_g[None, :], Wv * kvln_g[None, :]
    bq_f, bk_f, bv_f = Wq @ qln_b, Wk @ kvln_b, Wv @ kvln_b
    bg_f = Wg @ qln_b + bg
    idm = np.eye(128, dtype=np.float32)

    if _NC_CACHE[0] is None:
        _NC_CACHE[0] = _build()
    nc = _NC_CACHE[0]

    xT = {}
    for beta in range(B):
        for nm, x in (("q", q), ("k", k), ("v", v)):
            xT[nm, beta] = np.ascontiguousarray(x[beta].T).astype(bf16)

    in_maps = []
    for c in range(NC):
        beta, g = c // GPC, c % GPC
        m0 = MPC * g
        sl = slice(m0, m0 + MPC)
        in_maps.append({
            "xqT": xT["q", beta], "xkT": xT["k", beta], "xvT": xT["v", beta],
            "wqT": Wqp[sl, :].T.astype(bf16), "wkT": Wkp[sl, :].T.astype(bf16),
            "wvT": Wvp[sl, :].T.astype(bf16), "wgT": Wgp[sl, :].T.astype(bf16),
            "woT": Wo[:, sl].T.astype(bf16),
            "mucq": -Wqp[sl, :].sum(1)[None, :].astype(bf16),
            "muck": -Wkp[sl, :].sum(1)[None, :].astype(bf16),
            "mucv": -Wvp[sl, :].sum(1)[None, :].astype(bf16),
            "mucg": -Wgp[sl, :].sum(1)[None, :].astype(bf16),
            "bq": bq_f[sl], "bk": bk_f[sl], "bgt": bg_f[sl],
            "bvb": np.ascontiguousarray(
                np.broadcast_to(bv_f[sl][None, :], (128, MPC))).astype(np.float32),
            "identf": idm,
        })
    global _last_in_maps
    _last_in_maps = in_maps
    res = bass_utils.run_bass_kernel_spmd(nc, in_maps, core_ids=list(range(NC)))
    out = np.empty((B, LQ, D), np.float32)
    for beta in range(B):
        rowsT = np.empty((D, LQ), np.float32)
        for g in range(GPC):
            rowsT[MPC * g:MPC * (g + 1)] = \
                res.results[GPC * beta + g]["out"].astype(np.float32)
        out[beta] = rowsT.T
    return out
